# revision 1
# baseline (speedup 1.0000x reference)
"""AMIP router kernel for 8 TRN2 NeuronCores.

Sharding: data-parallel over tokens. B=4 batches x M=1024 masked tokens
= 4096 tokens; core c handles 512 tokens (half of batch c//2's masked set).
Router/expert weights are replicated (streamed from each core's HBM).

Device math (all matmuls bf16 with f32 PSUM accumulation):
  phase A : h_avgT[d,t] = sum_s hLw[s,d] * ATw[s,t] per 128-token chunk,
            where hLw/ATw are per-chunk windows of h_L and the averaging
            matrix (tokens are sorted, so a 128-token chunk only touches a
            ~WIN-row band of the sequence; host shifts each window to start
            at row 0 so the SPMD graph stays static).
  gate    : expT[k,t] = exp(W_r^T h_mask + b_r)          -- softmax denominator
            cancels in the final LayerNorm (scale invariance), so the gate is
            left unnormalized.
  phase B : H_T[h,t] = gelu(W1k^T X + b1k), Hg = H * g~  (X = [h_avg | h_mask])
  phase C : Y[t,d] = sum_k Hg_k^T W2k + exp^T b2, accumulated in PSUM,
            then LayerNorm over d and DMA out.

Host does only integer index prep, dtype casts, sharding and the final
scatter of LN rows into the zero output (tokens with no valid neighbors
keep zero rows).

Hardware notes (found the hard way, kept for future iterations):
- vector.tensor_tensor_reduce passes CoreSim and walrus but crashes the
  exec unit on silicon (NRT_EXEC_UNIT_UNRECOVERABLE) -- tried both
  in0==in1 and distinct SBUF/PSUM operands. Keep sum-of-squares on the
  ACT engine (Square + accum_out).
- DMA issue costs ~0.57us of SP-sequencer time per descriptor; packing
  many small loads into few wide DMAs is what keeps the DMA-bound front
  of this kernel fed. hLw/ATw ride the ACT-sequencer HWDGE ring so they
  do not queue behind the W1 stream on the SP ring.
"""

import os
import numpy as np
import ml_dtypes

import concourse.bass as bass
import concourse.bacc as bacc
import concourse.tile as tile
import concourse.mybir as mybir
from concourse.bass_utils import run_bass_kernel_spmd

BF16 = mybir.dt.bfloat16
F32 = mybir.dt.float32
AF = mybir.ActivationFunctionType
ALU = mybir.AluOpType

B, S, D, K = 4, 2048, 4096, 8
M = S // 2
NCORES = 8
TOK = B * M // NCORES          # 512 tokens per core
DH = D // 4                    # 1024 expert hidden
TD = 2 * D                     # 8192 expert input
NT = TOK // 128                # 4 token chunks
NF = TD // 128                 # 64 X-feature chunks
ND = D // 128                  # 32 output-dim chunks
NH = DH // 128                 # 8 hidden chunks

_NC_CACHE = {}
_LAST_WIN = 384


def _build_nc(win):
    nw = win // 128
    nc = bacc.Bacc("TRN2", target_bir_lowering=False, debug=False,
                   num_devices=NCORES)

    hLw = nc.dram_tensor("hLw", [NT, win, D], BF16, kind="ExternalInput")
    ATw = nc.dram_tensor("ATw", [128, NT * nw * 128], BF16, kind="ExternalInput")
    hmT = nc.dram_tensor("hmT", [128, ND * TOK], BF16, kind="ExternalInput")
    W1 = nc.dram_tensor("W1", [K, TD, DH], BF16, kind="ExternalInput")
    W2 = nc.dram_tensor("W2", [K, DH, D], BF16, kind="ExternalInput")
    Wr = nc.dram_tensor("Wr", [128, ND * K], BF16, kind="ExternalInput")
    br = nc.dram_tensor("br", [K, 1], F32, kind="ExternalInput")
    b1 = nc.dram_tensor("b1", [128, K * NH], F32, kind="ExternalInput")
    b2 = nc.dram_tensor("b2", [K, D], BF16, kind="ExternalInput")
    sel = nc.dram_tensor("sel", [K, K * 128], BF16, kind="ExternalInput")
    out = nc.dram_tensor("out", [TOK, D], F32, kind="ExternalOutput")

    act_fn = (AF.Relu if os.environ.get("AMIP_ACT") == "relu" else AF.Gelu)

    with tile.TileContext(nc) as tc:
        with (
            tc.tile_pool(name="hgt", bufs=K * NH) as p_hgt,
            tc.tile_pool(name="small", bufs=1) as p_small,
        ):
            # ---- small constants
            wr_sb = p_small.tile([128, ND * K], BF16)
            nc.sync.dma_start(wr_sb[:], Wr[:, :])
            br_sb = p_small.tile([K, 1], F32)
            nc.sync.dma_start(br_sb[:], br[:, :])
            b1_sb = p_small.tile([128, K * NH], F32)
            nc.sync.dma_start(b1_sb[:], b1[:, :])
            b2_sb = p_small.tile([K, D], BF16)
            nc.sync.dma_start(b2_sb[:], b2[:, :])
            sel_sb = p_small.tile([K, K * 128], BF16)
            nc.sync.dma_start(sel_sb[:], sel[:, :])
            expT = p_small.tile([K, TOK], BF16)
            eps_sb = p_small.tile([128, 1], F32)
            nc.gpsimd.memset(eps_sb[:], 1e-5)

            hgt = [None] * (K * NH)

            with tc.tile_pool(name="xt", bufs=NF) as p_xt:
                xt = [None] * NF

                # ---- XT lower half: h_maskT as one packed DMA
                # (host supplies hmT pre-packed as [128, ND*TOK], chunk i in
                # columns [i*TOK, (i+1)*TOK))
                hm_tile = p_xt.tile([128, ND * TOK], BF16, name="xtm",
                                    tag="xtm", bufs=1)
                for p in range(16):
                    w = ND * TOK // 16
                    nc.sync.dma_start(hm_tile[:, p * w:(p + 1) * w],
                                      hmT[:, p * w:(p + 1) * w])
                for f in range(ND, NF):
                    xt[f] = hm_tile[:, (f - ND) * TOK:(f - ND + 1) * TOK]

                # ---- gate: logitsT[k,t] accumulated over D, then exp
                with tc.tile_pool(name="psG", bufs=1, space="PSUM") as psG:
                    ps_g = psG.tile([K, TOK], F32)
                    for i in range(ND):
                        nc.tensor.matmul(
                            ps_g[:], wr_sb[:, i * K:(i + 1) * K], xt[ND + i][:],
                            start=(i == 0), stop=(i == ND - 1))
                    nc.scalar.activation(expT[:], ps_g[:], AF.Exp,
                                         bias=br_sb[:, 0:1])

                # ---- broadcast each gate row to all 128 partitions via a
                # one-hot selector matmul (gb_all[k][p, t] = expT[k, t])
                gb_all = []
                with tc.tile_pool(name="psGB", bufs=2, space="PSUM") as psGB:
                    for k in range(K):
                        pg = psGB.tile([128, TOK], F32, name="pgb", tag="pgb")
                        nc.tensor.matmul(pg[:], sel_sb[:, k * 128:(k + 1) * 128],
                                         expT[:], start=True, stop=True)
                        gb = p_hgt.tile([128, TOK], BF16, name="gball",
                                        tag="gball", bufs=K)
                        nc.scalar.copy(gb[:], pg[:])
                        gb_all.append(gb)

                # ---- expert-0 h_mask-half pre-pass: accumulate the
                # f in [ND, NF) half of expert 0's hidden pre-activation
                # while the phase-A windows are still streaming in (fills
                # the DMA-bound front with PE work). Partials are combined
                # with the top half inside phase B.
                NPRE = 2
                b0_partial = [[] for _ in range(NPRE)]
                with (
                    tc.tile_pool(name="w1s0", bufs=8) as p_w10,
                    tc.tile_pool(name="psB0", bufs=8, space="PSUM") as psB0,
                ):
                    for kp in range(NPRE):
                        pst0 = [psB0.tile([128, TOK], F32, name="psb0",
                                          tag="psb0") for _ in range(NH)]
                        for f in range(ND, NF):
                            slab = p_w10.tile([128, DH], BF16, name="w10",
                                              tag="w10")
                            nc.sync.dma_start(
                                slab[:], W1[kp, f * 128:(f + 1) * 128, :])
                            for h in range(NH):
                                nc.tensor.matmul(
                                    pst0[h][:], slab[:, h * 128:(h + 1) * 128],
                                    xt[f][:], start=(f == ND),
                                    stop=(f == NF - 1))
                        for h in range(NH):
                            t = p_xt.tile([128, TOK], BF16, name="b0p",
                                          tag="b0p", bufs=NPRE * NH)
                            nc.vector.tensor_copy(t[:], pst0[h][:])
                            b0_partial[kp].append(t)

                # ---- phase A: windowed h_avgT into XT upper half.
                # One PSUM bank holds all 4 token-quarters of a d-chunk
                # (independent accumulation groups per 128-col slice), so
                # each d-chunk drains with a single ACT copy.
                with (
                    tc.tile_pool(name="atw", bufs=NT * nw) as p_at,
                    tc.tile_pool(name="hlw", bufs=10) as p_hl,
                    tc.tile_pool(name="psA", bufs=8, space="PSUM") as psA,
                ):
                    at_tile = p_at.tile([128, NT * nw * 128], BF16,
                                        name="atw", tag="atw", bufs=1)
                    nc.scalar.dma_start(at_tile[:], ATw[:, :])
                    atw = [at_tile[:, q * 128:(q + 1) * 128]
                           for q in range(NT * nw)]
                    for dcg in range(ND // 8):    # groups of 8 d-chunks
                        pts = [psA.tile([128, TOK], F32, name="psa",
                                        tag="psa") for _ in range(8)]
                        for i in range(NT):
                            for s in range(nw):
                                slab = p_hl.tile([128, 1024], BF16)
                                nc.scalar.dma_start(
                                    slab[:], hLw[i, s * 128:(s + 1) * 128,
                                                 dcg * 1024:(dcg + 1) * 1024])
                                for j in range(8):
                                    nc.tensor.matmul(
                                        pts[j][:, i * 128:(i + 1) * 128],
                                        slab[:, j * 128:(j + 1) * 128],
                                        atw[i * nw + s][:],
                                        start=(s == 0), stop=(s == nw - 1))
                        for j in range(8):
                            t = p_xt.tile([128, TOK], BF16, name="xta",
                                          tag="xta", bufs=ND)
                            if j % 2 == 0:
                                nc.scalar.copy(t[:], pts[j][:])
                            else:
                                nc.vector.tensor_copy(t[:], pts[j][:])
                            xt[dcg * 8 + j] = t

                # ---- phase B: per-expert hidden, gelu, gate multiply.
                # Expert 0 only accumulates the h_avg half here; the
                # pre-pass partial is folded in before the activation.
                with (
                    tc.tile_pool(name="w1s", bufs=12) as p_w1,
                    tc.tile_pool(name="htmp", bufs=2) as p_h,
                    tc.tile_pool(name="psB", bufs=8, space="PSUM") as psB,
                ):
                    for k in range(K):
                        nf_hi = ND if k < NPRE else NF
                        pst = [psB.tile([128, TOK], F32, name="psb",
                                        tag="psb") for _ in range(NH)]
                        for f in range(nf_hi):
                            slab = p_w1.tile([128, DH], BF16)
                            nc.sync.dma_start(
                                slab[:], W1[k, f * 128:(f + 1) * 128, :])
                            for h in range(NH):
                                nc.tensor.matmul(
                                    pst[h][:], slab[:, h * 128:(h + 1) * 128],
                                    xt[f][:], start=(f == 0),
                                    stop=(f == nf_hi - 1))
                        for h in range(NH):
                            if k < NPRE:
                                nc.vector.tensor_add(pst[h][:], pst[h][:],
                                                     b0_partial[k][h][:])
                            ht = p_h.tile([128, TOK], BF16)
                            col = k * NH + h
                            nc.scalar.activation(ht[:], pst[h][:], act_fn,
                                                 bias=b1_sb[:, col:col + 1])
                            g = p_hgt.tile([128, TOK], BF16)
                            nc.vector.tensor_mul(g[:], ht[:], gb_all[k][:])
                            hgt[col] = g
            # p_xt released: phase C's Y tiles reuse its address space.

            # ---- phase C: Y = sum_k Hg_k^T @ W2k + expT^T @ b2; LayerNorm.
            # Two token-half passes over W2: the first half's LayerNorm +
            # output DMA overlap the second half's matmuls.
            with (
                tc.tile_pool(name="w2s", bufs=14) as p_w2,
                tc.tile_pool(name="ysb", bufs=NT) as p_y,
                tc.tile_pool(name="sq", bufs=2) as p_sq,
                tc.tile_pool(name="stat", bufs=16) as p_stat,
                tc.tile_pool(name="psC", bufs=8, space="PSUM") as psC,
            ):
                ysb = [p_y.tile([128, D], F32, name="ysb", tag="ysb")
                       for _ in range(NT)]
                sums = [p_stat.tile([128, 8], F32, name="sums", tag="sums")
                        for _ in range(NT)]
                sumsq = [p_stat.tile([128, 8], F32, name="sumsq", tag="sumsq")
                         for _ in range(NT)]
                skip_ln = os.environ.get("AMIP_ABLATE") == "noln"
                inv_d = 1.0 / D
                half = D // 2
                for tp in range(2):
                  tset = list(range(tp * 2, tp * 2 + 2))
                  w2_n = 0
                  for dcp in range(4):            # 1024-wide d column pairs
                    pst = {}
                    for u in range(2):
                        for t in tset:
                            pst[u * NT + t] = psC.tile([128, 512], F32,
                                                       name="psc", tag="psc")
                    for k in range(K):
                        for h in range(NH):
                            slab = p_w2.tile([128, 1024], BF16)
                            w2_n += 1
                            nc.sync.dma_start(
                                slab[:], W2[k, h * 128:(h + 1) * 128,
                                            dcp * 1024:(dcp + 1) * 1024])
                            first = (k == 0 and h == 0)
                            for u in range(2):
                                for t in tset:
                                    nc.tensor.matmul(
                                        pst[u * NT + t][:],
                                        hgt[k * NH + h][:, t * 128:(t + 1) * 128],
                                        slab[:, u * 512:(u + 1) * 512],
                                        start=first, stop=False)
                    for u in range(2):
                        dc = dcp * 2 + u
                        for t in tset:
                            nc.tensor.matmul(
                                pst[u * NT + t][:], expT[:, t * 128:(t + 1) * 128],
                                b2_sb[:, dc * 512:(dc + 1) * 512],
                                start=False, stop=True)
                    for u in range(2):
                        dc = dcp * 2 + u
                        for t in tset:
                            p = pst[u * NT + t]
                            nc.scalar.activation(
                                ysb[t][:, dc * 512:(dc + 1) * 512], p[:],
                                AF.Identity, accum_out=sums[t][:, dc:dc + 1])
                            sq = p_sq.tile([128, 512], F32)
                            nc.scalar.activation(
                                sq[:], p[:], AF.Square,
                                accum_out=sumsq[t][:, dc:dc + 1])

                  # ---- LayerNorm rows for this token half, then output.
                  # Apply is split: ACT does the left half, DVE the right.
                  for t in tset:
                    if skip_ln:
                        nc.sync.dma_start(out[t * 128:(t + 1) * 128, :],
                                          ysb[t][:])
                        continue
                    s1 = p_stat.tile([128, 1], F32)
                    nc.vector.tensor_reduce(s1[:], sums[t][:, :],
                                            mybir.AxisListType.X, ALU.add)
                    s2 = p_stat.tile([128, 1], F32)
                    nc.vector.tensor_reduce(s2[:], sumsq[t][:, :],
                                            mybir.AxisListType.X, ALU.add)
                    mu = p_stat.tile([128, 1], F32)
                    nc.vector.tensor_scalar_mul(mu[:], s1[:], inv_d)
                    ex2 = p_stat.tile([128, 1], F32)
                    nc.vector.tensor_scalar_mul(ex2[:], s2[:], inv_d)
                    musq = p_stat.tile([128, 1], F32)
                    nc.vector.tensor_mul(musq[:], mu[:], mu[:])
                    var = p_stat.tile([128, 1], F32)
                    nc.vector.tensor_sub(var[:], ex2[:], musq[:])
                    std = p_stat.tile([128, 1], F32)
                    nc.scalar.activation(std[:], var[:], AF.Sqrt,
                                         bias=eps_sb[:, 0:1])
                    rstd = p_stat.tile([128, 1], F32)
                    nc.vector.reciprocal(rstd[:], std[:])
                    nmr = p_stat.tile([128, 1], F32)
                    nc.vector.tensor_mul(nmr[:], mu[:], rstd[:])
                    nc.vector.tensor_scalar_mul(nmr[:], nmr[:], -1.0)
                    # left half on ACT: y*rstd - mu*rstd
                    nc.scalar.activation(ysb[t][:, :half], ysb[t][:, :half],
                                         AF.Identity, bias=nmr[:, 0:1],
                                         scale=rstd[:, 0:1])
                    nc.scalar.dma_start(out[t * 128:(t + 1) * 128, :half],
                                          ysb[t][:, :half])
                    # right half on DVE: (y - mu) * rstd
                    nc.vector.tensor_scalar(ysb[t][:, half:], ysb[t][:, half:],
                                            mu[:], rstd[:],
                                            ALU.subtract, ALU.mult)
                    nc.sync.dma_start(out[t * 128:(t + 1) * 128, half:],
                                      ysb[t][:, half:])

    nc.compile()
    return nc


def get_nc(win=384):
    if win not in _NC_CACHE:
        _NC_CACHE[win] = _build_nc(win)
    return _NC_CACHE[win]


def _host_prep(h_L, W_r, b_r, W1, b1, W2, b2, mask_indices, unmasked_indices,
               range_r):
    """Integer-index prep + dtype casts + sharding. Returns (win, in_maps,
    scatter plans)."""
    r = int(range_r)
    bf = ml_dtypes.bfloat16

    is_un = np.zeros((B, S), bool)
    is_un[np.arange(B)[:, None], unmasked_indices] = True
    if r > 0:
        offs = np.concatenate([np.arange(-r, 0), np.arange(1, r + 1)])
        pos = mask_indices[:, :, None] + offs[None, None, :]      # [B,M,2r]
        inb = (pos >= 0) & (pos < S)
        posc = np.clip(pos, 0, S - 1)
        valid = inb & is_un[np.arange(B)[:, None, None], posc]
        cnt = valid.sum(-1)
        w = (1.0 / np.maximum(cnt, 1)).astype(np.float32)
    else:
        cnt = np.zeros((B, M), np.int64)

    W1b = np.ascontiguousarray(W1).astype(bf)
    W2b = np.ascontiguousarray(W2).astype(bf)
    b2b = np.ascontiguousarray(b2).astype(bf)
    # Wr rearranged so chunk dc lives in columns [dc*K, (dc+1)*K)
    Wrb = np.ascontiguousarray(
        W_r.reshape(ND, 128, K).transpose(1, 0, 2).reshape(128, ND * K)
    ).astype(bf)
    brf = np.ascontiguousarray(b_r.reshape(K, 1)).astype(np.float32)
    # b1 col k*NH+h = b1[k, h*128:(h+1)*128]
    b1f = np.ascontiguousarray(
        b1.reshape(K, NH, 128).transpose(2, 0, 1).reshape(128, K * NH)
    ).astype(np.float32)

    hLb = [np.ascontiguousarray(h_L[b]).astype(bf) for b in range(B)]

    selb = np.zeros((K, K * 128), bf)
    for k in range(K):
        selb[k, k * 128:(k + 1) * 128] = 1

    per_batch = M // (NCORES // B)            # 512 tokens per core
    # window size: max span of any 128-token chunk's neighbor band
    win = 256
    for b in range(B):
        for c in range(NCORES // B):
            toks = mask_indices[b, c * per_batch:(c + 1) * per_batch]
            for i in range(NT):
                ch = toks[i * 128:(i + 1) * 128]
                span = int(ch[-1]) + r - (int(ch[0]) - r) + 1
                win = max(win, -(-span // 128) * 128)
    win = min(win, -(-S // 128) * 128)

    in_maps = []
    plans = []
    for c in range(NCORES):
        b = c // (NCORES // B)
        t0 = (c % (NCORES // B)) * per_batch
        toks = mask_indices[b, t0:t0 + per_batch]
        hmTf = np.ascontiguousarray(h_L[b][toks].T)      # [D, TOK]
        hmTc = np.ascontiguousarray(
            hmTf.reshape(ND, 128, TOK).transpose(1, 0, 2).reshape(
                128, ND * TOK)).astype(bf)
        nw = win // 128
        hLwc = np.zeros((NT, win, D), bf)
        ATwc = np.zeros((NT, win, 128), np.float32)
        for i in range(NT):
            ch = toks[i * 128:(i + 1) * 128]
            w0 = min(max(int(ch[0]) - r, 0), S - win)
            hLwc[i] = hLb[b][w0:w0 + win]
            if r > 0:
                mrow = t0 + i * 128
                v = valid[b, mrow:mrow + 128]            # [128, 2r]
                pc = posc[b, mrow:mrow + 128]
                jj, oo = np.nonzero(v)
                ATwc[i, pc[jj, oo] - w0, jj] = w[b, mrow + jj]
        ATwp = np.ascontiguousarray(
            ATwc.reshape(NT, nw, 128, 128).transpose(2, 0, 1, 3).reshape(
                128, NT * nw * 128)).astype(bf)
        in_maps.append({
            "hLw": hLwc, "ATw": ATwp, "hmT": hmTc,
            "W1": W1b, "W2": W2b, "Wr": Wrb, "br": brf, "b1": b1f, "b2": b2b,
            "sel": selb,
        })
        plans.append((b, toks, cnt[b, t0:t0 + per_batch] > 0))
    return win, in_maps, plans


def kernel(h_L, W_r, b_r, W1, b1, W2, b2, mask_indices, unmasked_indices,
           range_r):
    h_L = np.asarray(h_L, np.float32)
    mask_indices = np.asarray(mask_indices)
    unmasked_indices = np.asarray(unmasked_indices)
    assert h_L.shape == (B, S, D) and mask_indices.shape == (B, M)

    win, in_maps, plans = _host_prep(
        h_L, np.asarray(W_r, np.float32), np.asarray(b_r, np.float32),
        np.asarray(W1, np.float32), np.asarray(b1, np.float32),
        np.asarray(W2, np.float32), np.asarray(b2, np.float32),
        mask_indices, unmasked_indices, range_r)

    global _LAST_WIN
    _LAST_WIN = win
    nc = get_nc(win)
    try:
        res = run_bass_kernel_spmd(nc, in_maps, core_ids=list(range(NCORES)))
    except Exception:
        # transient device faults (e.g. NRT_EXEC_UNIT_UNRECOVERABLE) happen
        # rarely under the axon tunnel; one retry clears them
        import time as _time
        _time.sleep(5)
        res = run_bass_kernel_spmd(nc, in_maps, core_ids=list(range(NCORES)))

    full = np.zeros((B, S, D), np.float32)
    for c in range(NCORES):
        b, toks, has = plans[c]
        o = np.asarray(res.results[c]["out"], np.float32)
        full[b, toks[has]] = o[has]
    return full



# revision 10
# speedup vs baseline: 1.3647x; 1.3647x over previous
"""AMIP router kernel for 8 TRN2 NeuronCores — fp8 DoubleRow edition.

Sharding: data-parallel over tokens (as baseline): core c handles 512
tokens (half of batch c//2's masked set). Router/expert weights
replicated.

Speed comes from fp8e4 (e4m3) matmuls in MatmulPerfMode.DoubleRow: one
PE instruction contracts 2 k-tiles (256 rows) at 0.5 cycles per output
column. Accuracy is restored with hi/lo residual passes accumulated
into the same PSUM group:

  X @ W  ~=  Xq @ Wq  +  Xlo @ Wq  +  Xq @ Wlo
  (Xq = fp8(X), Xlo = fp8(X - Xq); W pre-scaled x64 so its fp8
   quantization stays out of the subnormal range; the 1/64 is folded
   into the PSUM-drain activation's scale.)

Device math per core (TOK=512 tokens):
  phase A : h_avgT windows (bf16 matmuls, unchanged from baseline),
            PSUM drained to Xq (ACT cast) + Xlo (DVE sub) fp8 tiles.
  gate    : expT[k,t] = exp(W_r^T h_mask + b_r) from bf16 h_mask
            (unnormalized; softmax denominator cancels in LayerNorm).
            Broadcast via one-hot selector matmul scaled by HGS=1/8 so
            the gated hidden stays inside fp8e4's +-240 range.
  phase B : per expert, 3 DoubleRow passes over 32 feature-chunk pairs
            into 8 PSUM banks; GELU drain (scale 1/64, bias b1) -> ht;
            DVE produces Hgq = fp8(ht*gb), Hgf = bf16(ht*gb),
            Hglo = fp8(Hgf - Hgq).
            Variant "w1mask" drops the W1lo pass on the h_avg feature
            half (13% of X variance -> ~1% extra noise, measured
            rel_err 1.6e-2 vs 8.7e-3 for "full"; gate is 2e-2).
  phase C : Y = sum_k Hg_k^T W2_k via 3 DoubleRow passes over 4
            DH-chunk pairs, + expT^T b2 (bf16, b2 host-scaled x8);
            drain scale 1/8; LayerNorm; bf16 output DMA (host casts
            back to f32).
  Mask-half pre-pass for the first NPRE experts fills the DMA-bound
  front (windows still streaming) with PE work, as in the baseline.

Hardware notes inherited from the baseline session:
- vector.tensor_tensor_reduce crashes the exec unit on silicon; keep
  sum-of-squares on ACT (Square + accum_out).
- transient NRT_EXEC_UNIT_UNRECOVERABLE faults under the axon tunnel
  clear on retry.
Validated on silicon this session: ACT f32/bf16->fp8e4 casts and DVE
mixed-dtype subtract are bit-exact vs ml_dtypes; DoubleRow with 3D
[128,2,N] APs matches the interpreter semantics.
"""

import os
import numpy as np
import ml_dtypes

import concourse.bass as bass
import concourse.bacc as bacc
import concourse.tile as tile
import concourse.mybir as mybir
from concourse.bass_utils import run_bass_kernel_spmd

BF16 = mybir.dt.bfloat16
F32 = mybir.dt.float32
FP8 = mybir.dt.float8e4
NPF8 = ml_dtypes.float8_e4m3
AF = mybir.ActivationFunctionType
ALU = mybir.AluOpType
DR = mybir.MatmulPerfMode.DoubleRow

B, S, D, K = 4, 2048, 4096, 8
M = S // 2
NCORES = 8
TOK = B * M // NCORES          # 512 tokens per core
DH = D // 4                    # 1024 expert hidden
TD = 2 * D                     # 8192 expert input
NT = TOK // 128                # 4 token chunks
NF = TD // 128                 # 64 X-feature chunks
ND = D // 128                  # 32 output-dim chunks
NH = DH // 128                 # 8 hidden chunks
NFP = NF // 2                  # 32 feature-chunk pairs
NHP = NH // 2                  # 4 hidden-chunk pairs
MFP0 = NFP // 2                # first mask-half feature pair (16)
NPRE = 2                       # experts pre-passed on the mask half

S1 = 64.0                      # W1 host prescale (pow2)
S2 = 64.0                      # W2 host prescale
HGS = 0.125                    # gate prescale (pow2), drain undoes it

_NC_CACHE = {}
_LAST_KEY = (384, "w1mask")


def _build_nc(win, variant):
    nw = win // 128
    full_lo = variant == "full"
    nlop = NFP if full_lo else NFP - MFP0   # W1lo pairs staged
    lo_off = 0 if full_lo else MFP0

    nc = bacc.Bacc("TRN2", target_bir_lowering=False, debug=False,
                   num_devices=NCORES)

    hLw = nc.dram_tensor("hLw", [NT, win, D], BF16, kind="ExternalInput")
    ATw = nc.dram_tensor("ATw", [128, NT * nw * 128], BF16, kind="ExternalInput")
    hmT = nc.dram_tensor("hmT", [128, ND * TOK], BF16, kind="ExternalInput")
    W1q = nc.dram_tensor("W1q", [K, NFP, 128, 2, DH], FP8, kind="ExternalInput")
    W1l = nc.dram_tensor("W1l", [K, nlop, 128, 2, DH], FP8, kind="ExternalInput")
    W2q = nc.dram_tensor("W2q", [K, NHP, 128, 2, D], FP8, kind="ExternalInput")
    W2l = nc.dram_tensor("W2l", [K, NHP, 128, 2, D], FP8, kind="ExternalInput")
    Wr = nc.dram_tensor("Wr", [128, ND * K], BF16, kind="ExternalInput")
    br = nc.dram_tensor("br", [K, 1], F32, kind="ExternalInput")
    b1 = nc.dram_tensor("b1", [128, K * NH], F32, kind="ExternalInput")
    b2 = nc.dram_tensor("b2", [K, D], BF16, kind="ExternalInput")
    sel = nc.dram_tensor("sel", [K, K * 128], BF16, kind="ExternalInput")
    out = nc.dram_tensor("out", [TOK, D], BF16, kind="ExternalOutput")

    debug = os.environ.get("AMIP_DEBUG") == "1"
    if debug:
        xq_d = nc.dram_tensor("xq_d", [128, NF, TOK], FP8, kind="ExternalOutput")
        xlo_d = nc.dram_tensor("xlo_d", [128, NF, TOK], FP8, kind="ExternalOutput")
        expT_d = nc.dram_tensor("expT_d", [K, TOK], BF16, kind="ExternalOutput")
        hgq_d = nc.dram_tensor("hgq_d", [128, K * NH, TOK], FP8,
                               kind="ExternalOutput")
        hgl_d = nc.dram_tensor("hgl_d", [128, K * NH, TOK], FP8,
                               kind="ExternalOutput")
        yraw_d = nc.dram_tensor("yraw_d", [TOK, D], BF16, kind="ExternalOutput")

    inv_s1 = 1.0 / S1
    inv_cdrain = 1.0 / (S2 * HGS)

    with tile.TileContext(nc) as tc:
        with (
            tc.tile_pool(name="small", bufs=1) as p_small,
            tc.tile_pool(name="hg", bufs=1) as p_hg,
        ):
            # ---- small constants
            wr_sb = p_small.tile([128, ND * K], BF16)
            nc.sync.dma_start(wr_sb[:], Wr[:, :])
            br_sb = p_small.tile([K, 1], F32)
            nc.sync.dma_start(br_sb[:], br[:, :])
            b1_sb = p_small.tile([128, K * NH], F32)
            nc.sync.dma_start(b1_sb[:], b1[:, :])
            b2_sb = p_small.tile([K, D], BF16)
            nc.sync.dma_start(b2_sb[:], b2[:, :])
            sel_sb = p_small.tile([K, K * 128], BF16)
            nc.sync.dma_start(sel_sb[:], sel[:, :])
            expT = p_small.tile([K, TOK], BF16)
            eps_sb = p_small.tile([128, 1], F32)
            nc.gpsimd.memset(eps_sb[:], 1e-5)

            # gated-hidden fp8 tiles, chunk-major so DoubleRow pairs are
            # adjacent: column block k*NH+h holds expert k's hidden chunk h
            hgq = p_hg.tile([128, K * NH, TOK], FP8)
            hgl = p_hg.tile([128, K * NH, TOK], FP8)
            gb_all = []

            with tc.tile_pool(name="xt", bufs=1) as p_xt:
                xq = p_xt.tile([128, NF, TOK], FP8)
                xlo = p_xt.tile([128, NF, TOK], FP8)

                with tc.tile_pool(name="hmp", bufs=1) as p_hm:
                    hm = p_hm.tile([128, ND * TOK], BF16)
                    for p in range(16):
                        w = ND * TOK // 16
                        nc.sync.dma_start(hm[:, p * w:(p + 1) * w],
                                          hmT[:, p * w:(p + 1) * w])

                    # ---- gate: logitsT[k,t] over D, then exp (bf16 h_mask)
                    with tc.tile_pool(name="psG", bufs=1, space="PSUM") as psG:
                        ps_g = psG.tile([K, TOK], F32)
                        for i in range(ND):
                            nc.tensor.matmul(
                                ps_g[:], wr_sb[:, i * K:(i + 1) * K],
                                hm[:, i * TOK:(i + 1) * TOK],
                                start=(i == 0), stop=(i == ND - 1))
                        nc.scalar.activation(expT[:], ps_g[:], AF.Exp,
                                             bias=br_sb[:, 0:1])

                    # ---- gate broadcast (sel entries = HGS, so gb = exp/8)
                    with tc.tile_pool(name="psGB", bufs=2, space="PSUM") as psGB:
                        for k in range(K):
                            pg = psGB.tile([128, TOK], F32, name="pgb",
                                           tag="pgb")
                            nc.tensor.matmul(pg[:],
                                             sel_sb[:, k * 128:(k + 1) * 128],
                                             expT[:], start=True, stop=True)
                            gb = p_hg.tile([128, TOK], BF16, name="gball",
                                           tag="gball", bufs=K)
                            nc.scalar.copy(gb[:], pg[:])
                            gb_all.append(gb)

                    # ---- h_mask half of X: fp8 cast + residual
                    for i in range(ND):
                        src = hm[:, i * TOK:(i + 1) * TOK]
                        nc.scalar.copy(xq[:, ND + i, :], src)
                        nc.vector.tensor_sub(xlo[:, ND + i, :], src,
                                             xq[:, ND + i, :])
                # hm freed

                # ---- mask-half pre-pass for experts < NPRE (fills the
                # DMA-bound front with PE work); all 3 passes.
                b0_partial = [[] for _ in range(NPRE)]
                with (
                    tc.tile_pool(name="w1s0", bufs=8) as p_w10,
                    tc.tile_pool(name="psB0", bufs=8, space="PSUM") as psB0,
                ):
                    for kp in range(NPRE):
                        pst0 = [psB0.tile([128, TOK], F32, name="psb0",
                                          tag="psb0") for _ in range(NH)]
                        for fp in range(MFP0, NFP):
                            slq = p_w10.tile([128, 2, DH], FP8, name="w10q",
                                             tag="w10")
                            nc.sync.dma_start(slq[:], W1q[kp, fp, :, :, :])
                            sll = p_w10.tile([128, 2, DH], FP8, name="w10l",
                                             tag="w10")
                            nc.sync.dma_start(sll[:], W1l[kp, fp - lo_off, :, :, :])
                            first = fp == MFP0
                            last = fp == NFP - 1
                            # full-bank-width matmuls: exactly one start=True
                            # per PSUM bank (start poisons the whole 2KB bank
                            # on silicon, so sub-bank regions must not
                            # interleave accumulation groups)
                            for h in range(NH):
                                lq = slq[:, :, h * 128:(h + 1) * 128]
                                ll = sll[:, :, h * 128:(h + 1) * 128]
                                o = pst0[h][:]
                                rq = xq[:, 2 * fp:2 * fp + 2, :]
                                rl = xlo[:, 2 * fp:2 * fp + 2, :]
                                nc.tensor.matmul(o, lq, rq, perf_mode=DR,
                                                 start=first, stop=False)
                                nc.tensor.matmul(o, lq, rl, perf_mode=DR,
                                                 start=False, stop=False)
                                nc.tensor.matmul(o, ll, rq, perf_mode=DR,
                                                 start=False, stop=last)
                        for h in range(NH):
                            t = p_xt.tile([128, TOK], BF16, name="b0p",
                                          tag="b0p", bufs=NPRE * NH)
                            nc.vector.tensor_copy(t[:], pst0[h][:])
                            b0_partial[kp].append(t)

                # ---- phase A: windowed h_avgT -> Xq/Xlo fp8 (bf16 matmuls)
                with (
                    tc.tile_pool(name="atw", bufs=1) as p_at,
                    tc.tile_pool(name="hlw", bufs=10) as p_hl,
                    tc.tile_pool(name="psA", bufs=8, space="PSUM") as psA,
                ):
                    at_tile = p_at.tile([128, NT * nw * 128], BF16)
                    nc.scalar.dma_start(at_tile[:], ATw[:, :])
                    atw = [at_tile[:, q * 128:(q + 1) * 128]
                           for q in range(NT * nw)]
                    for dcg in range(ND // 8):    # groups of 8 d-chunks
                        pts = [psA.tile([128, TOK], F32, name="psa",
                                        tag="psa") for _ in range(8)]
                        for i in range(NT):
                            for s in range(nw):
                                slab = p_hl.tile([128, 1024], BF16)
                                nc.scalar.dma_start(
                                    slab[:], hLw[i, s * 128:(s + 1) * 128,
                                                 dcg * 1024:(dcg + 1) * 1024])
                                for j in range(8):
                                    nc.tensor.matmul(
                                        pts[j][:, i * 128:(i + 1) * 128],
                                        slab[:, j * 128:(j + 1) * 128],
                                        atw[i * nw + s][:],
                                        start=(s == 0), stop=(s == nw - 1))
                        for j in range(8):
                            dc = dcg * 8 + j
                            nc.scalar.copy(xq[:, dc, :], pts[j][:])
                            nc.vector.tensor_sub(xlo[:, dc, :], pts[j][:],
                                                 xq[:, dc, :])

                # ---- phase B main: per-expert 3-pass DoubleRow + drain
                with (
                    tc.tile_pool(name="w1s", bufs=10) as p_w1,
                    tc.tile_pool(name="htmp", bufs=4) as p_h,
                    tc.tile_pool(name="psB", bufs=8, space="PSUM") as psB,
                ):
                    for k in range(K):
                        pst = [psB.tile([128, TOK], F32, name="psb",
                                        tag="psb") for _ in range(NH)]
                        fp_hi = MFP0 if k < NPRE else NFP
                        for fp in range(fp_hi):
                            slq = p_w1.tile([128, 2, DH], FP8, name="w1q",
                                            tag="w1")
                            nc.sync.dma_start(slq[:], W1q[k, fp, :, :, :])
                            has_lo = full_lo or fp >= MFP0
                            if has_lo:
                                sll = p_w1.tile([128, 2, DH], FP8, name="w1l",
                                                tag="w1")
                                nc.sync.dma_start(sll[:],
                                                  W1l[k, fp - lo_off, :, :, :])
                            first = fp == 0
                            last = fp == fp_hi - 1
                            for h in range(NH):
                                lq = slq[:, :, h * 128:(h + 1) * 128]
                                o = pst[h][:]
                                rq = xq[:, 2 * fp:2 * fp + 2, :]
                                rl = xlo[:, 2 * fp:2 * fp + 2, :]
                                nc.tensor.matmul(o, lq, rq, perf_mode=DR,
                                                 start=first, stop=False)
                                nc.tensor.matmul(
                                    o, lq, rl, perf_mode=DR, start=False,
                                    stop=(last and not has_lo))
                                if has_lo:
                                    ll = sll[:, :, h * 128:(h + 1) * 128]
                                    nc.tensor.matmul(o, ll, rq,
                                                     perf_mode=DR,
                                                     start=False,
                                                     stop=last)
                        for h in range(NH):
                            if k < NPRE:
                                nc.vector.tensor_add(pst[h][:], pst[h][:],
                                                     b0_partial[k][h][:])
                            col = k * NH + h
                            ht = p_h.tile([128, TOK], BF16, name="ht",
                                          tag="ht")
                            nc.scalar.activation(ht[:], pst[h][:], AF.Gelu,
                                                 bias=b1_sb[:, col:col + 1],
                                                 scale=inv_s1)
                            nc.vector.tensor_mul(hgq[:, col, :], ht[:],
                                                 gb_all[k][:])
                            hgf = p_h.tile([128, TOK], BF16, name="hgf",
                                           tag="hgf")
                            nc.vector.tensor_mul(hgf[:], ht[:], gb_all[k][:])
                            nc.vector.tensor_sub(hgl[:, col, :], hgf[:],
                                                 hgq[:, col, :])
                if debug:
                    nc.sync.dma_start(xq_d[:, :, :], xq[:])
                    nc.sync.dma_start(xlo_d[:, :, :], xlo[:])
            # xq/xlo/b0 freed: phase C reuses their address space
            if debug:
                nc.sync.dma_start(expT_d[:, :], expT[:])
                nc.sync.dma_start(hgq_d[:, :, :], hgq[:])
                nc.sync.dma_start(hgl_d[:, :, :], hgl[:])

            # ---- phase C: Y = sum_k Hg_k^T W2_k (3-pass DR) + expT^T b2;
            # LayerNorm; bf16 out. Single sweep over W2 (d-groups of 1024).
            with (
                tc.tile_pool(name="w2s", bufs=8) as p_w2,
                tc.tile_pool(name="ysb", bufs=1) as p_y,
                tc.tile_pool(name="sq", bufs=4) as p_sq,
                tc.tile_pool(name="stat", bufs=16) as p_stat,
                tc.tile_pool(name="psC", bufs=8, space="PSUM") as psC,
            ):
                ysb = [p_y.tile([128, D], BF16, name="ysb", tag="ysbt",
                                bufs=NT) for _ in range(NT)]
                sums = [p_stat.tile([128, 8], F32, name="sums", tag="sums")
                        for _ in range(NT)]
                sumsq = [p_stat.tile([128, 8], F32, name="sumsq", tag="sumsq")
                         for _ in range(NT)]
                for dg in range(4):               # 1024-wide d column groups
                    pst = {}
                    # b2 bias term first: the sole start=True per bank, at
                    # full bank width (start poisons the whole 2KB bank)
                    for t in range(NT):
                        for u in range(2):
                            p = psC.tile([128, 512], F32, name="psc",
                                         tag="psc")
                            pst[t * 2 + u] = p
                            c0 = dg * 1024 + u * 512
                            nc.tensor.matmul(p[:],
                                             expT[:, t * 128:(t + 1) * 128],
                                             b2_sb[:, c0:c0 + 512],
                                             start=True, stop=False)
                    for k in range(K):
                        for hp in range(NHP):
                            slq = p_w2.tile([128, 2, 1024], FP8, name="w2q",
                                            tag="w2")
                            nc.scalar.dma_start(
                                slq[:], W2q[k, hp, :, :,
                                            dg * 1024:(dg + 1) * 1024])
                            sll = p_w2.tile([128, 2, 1024], FP8, name="w2l",
                                            tag="w2")
                            nc.scalar.dma_start(
                                sll[:], W2l[k, hp, :, :,
                                            dg * 1024:(dg + 1) * 1024])
                            last = k == K - 1 and hp == NHP - 1
                            col = k * NH + 2 * hp
                            for t in range(NT):
                                lq = hgq[:, col:col + 2,
                                         t * 128:(t + 1) * 128]
                                ll = hgl[:, col:col + 2,
                                         t * 128:(t + 1) * 128]
                                for u in range(2):
                                    o = pst[t * 2 + u][:]
                                    r = slice(u * 512, (u + 1) * 512)
                                    nc.tensor.matmul(
                                        o, lq, slq[:, :, r], perf_mode=DR,
                                        start=False, stop=False)
                                    nc.tensor.matmul(
                                        o, ll, slq[:, :, r], perf_mode=DR,
                                        start=False, stop=False)
                                    nc.tensor.matmul(
                                        o, lq, sll[:, :, r], perf_mode=DR,
                                        start=False, stop=last)
                    for t in range(NT):
                        for u in range(2):
                            dc = dg * 2 + u
                            p = pst[t * 2 + u]
                            nc.scalar.activation(
                                ysb[t][:, dc * 512:(dc + 1) * 512], p[:],
                                AF.Identity, scale=inv_cdrain,
                                accum_out=sums[t][:, dc:dc + 1])
                            sq = p_sq.tile([128, 512], F32, name="sq",
                                           tag="sq")
                            nc.scalar.activation(
                                sq[:], p[:], AF.Square, scale=inv_cdrain,
                                accum_out=sumsq[t][:, dc:dc + 1])

                if debug:
                    for t in range(NT):
                        nc.sync.dma_start(yraw_d[t * 128:(t + 1) * 128, :],
                                          ysb[t][:])
                # ---- LayerNorm rows, then output (bf16).
                inv_d = 1.0 / D
                half = D // 2
                for t in range(NT):
                    s1t = p_stat.tile([128, 1], F32)
                    nc.vector.tensor_reduce(s1t[:], sums[t][:, :],
                                            mybir.AxisListType.X, ALU.add)
                    s2t = p_stat.tile([128, 1], F32)
                    nc.vector.tensor_reduce(s2t[:], sumsq[t][:, :],
                                            mybir.AxisListType.X, ALU.add)
                    mu = p_stat.tile([128, 1], F32)
                    nc.vector.tensor_scalar_mul(mu[:], s1t[:], inv_d)
                    ex2 = p_stat.tile([128, 1], F32)
                    nc.vector.tensor_scalar_mul(ex2[:], s2t[:], inv_d)
                    musq = p_stat.tile([128, 1], F32)
                    nc.vector.tensor_mul(musq[:], mu[:], mu[:])
                    var = p_stat.tile([128, 1], F32)
                    nc.vector.tensor_sub(var[:], ex2[:], musq[:])
                    std = p_stat.tile([128, 1], F32)
                    nc.scalar.activation(std[:], var[:], AF.Sqrt,
                                         bias=eps_sb[:, 0:1])
                    rstd = p_stat.tile([128, 1], F32)
                    nc.vector.reciprocal(rstd[:], std[:])
                    nmr = p_stat.tile([128, 1], F32)
                    nc.vector.tensor_mul(nmr[:], mu[:], rstd[:])
                    nc.vector.tensor_scalar_mul(nmr[:], nmr[:], -1.0)
                    # left half on ACT: y*rstd - mu*rstd
                    nc.scalar.activation(ysb[t][:, :half], ysb[t][:, :half],
                                         AF.Identity, bias=nmr[:, 0:1],
                                         scale=rstd[:, 0:1])
                    nc.scalar.dma_start(out[t * 128:(t + 1) * 128, :half],
                                        ysb[t][:, :half])
                    # right half on DVE: (y - mu) * rstd
                    nc.vector.tensor_scalar(ysb[t][:, half:], ysb[t][:, half:],
                                            mu[:], rstd[:],
                                            ALU.subtract, ALU.mult)
                    nc.sync.dma_start(out[t * 128:(t + 1) * 128, half:],
                                      ysb[t][:, half:])

    nc.compile()
    return nc


def get_nc(win=384, variant=None):
    if variant is None:
        variant = os.environ.get("AMIP_VARIANT", "w1mask")
    key = (win, variant)
    if key not in _NC_CACHE:
        _NC_CACHE[key] = _build_nc(win, variant)
    return _NC_CACHE[key]


def _host_prep(h_L, W_r, b_r, W1, b1, W2, b2, mask_indices, unmasked_indices,
               range_r, variant):
    """Index prep + quantization + sharding."""
    r = int(range_r)
    bf = ml_dtypes.bfloat16
    full_lo = variant == "full"

    is_un = np.zeros((B, S), bool)
    is_un[np.arange(B)[:, None], unmasked_indices] = True
    if r > 0:
        offs = np.concatenate([np.arange(-r, 0), np.arange(1, r + 1)])
        pos = mask_indices[:, :, None] + offs[None, None, :]      # [B,M,2r]
        inb = (pos >= 0) & (pos < S)
        posc = np.clip(pos, 0, S - 1)
        valid = inb & is_un[np.arange(B)[:, None, None], posc]
        cnt = valid.sum(-1)
        w = (1.0 / np.maximum(cnt, 1)).astype(np.float32)
    else:
        cnt = np.zeros((B, M), np.int64)

    # W1 DoubleRow packs: [K, NFP, 128, 2, DH]
    W1s = np.ascontiguousarray(W1, np.float32) * np.float32(S1)
    W1q8 = W1s.astype(NPF8)
    W1l8 = (W1s - W1q8.astype(np.float32)).astype(NPF8)
    del W1s

    def pack_w1(a):
        return np.ascontiguousarray(
            a.reshape(K, NFP, 2, 128, DH).transpose(0, 1, 3, 2, 4))
    W1qp = pack_w1(W1q8)
    W1lp = pack_w1(W1l8) if full_lo else pack_w1(W1l8)[:, MFP0:]
    del W1q8, W1l8

    W2s = np.ascontiguousarray(W2, np.float32) * np.float32(S2)
    W2q8 = W2s.astype(NPF8)
    W2l8 = (W2s - W2q8.astype(np.float32)).astype(NPF8)
    del W2s

    def pack_w2(a):
        return np.ascontiguousarray(
            a.reshape(K, NHP, 2, 128, D).transpose(0, 1, 3, 2, 4))
    W2qp = pack_w2(W2q8)
    W2lp = pack_w2(W2l8)
    del W2q8, W2l8

    b2b = np.ascontiguousarray(b2 * np.float32(S2 * HGS)).astype(bf)
    Wrb = np.ascontiguousarray(
        W_r.reshape(ND, 128, K).transpose(1, 0, 2).reshape(128, ND * K)
    ).astype(bf)
    brf = np.ascontiguousarray(b_r.reshape(K, 1)).astype(np.float32)
    b1f = np.ascontiguousarray(
        b1.reshape(K, NH, 128).transpose(2, 0, 1).reshape(128, K * NH)
    ).astype(np.float32)

    hLb = [np.ascontiguousarray(h_L[b]).astype(bf) for b in range(B)]

    selb = np.zeros((K, K * 128), bf)
    for k in range(K):
        selb[k, k * 128:(k + 1) * 128] = np.float32(HGS)

    per_batch = M // (NCORES // B)            # 512 tokens per core
    win = 256
    for b in range(B):
        for c in range(NCORES // B):
            toks = mask_indices[b, c * per_batch:(c + 1) * per_batch]
            for i in range(NT):
                ch = toks[i * 128:(i + 1) * 128]
                span = int(ch[-1]) + r - (int(ch[0]) - r) + 1
                win = max(win, -(-span // 128) * 128)
    win = min(win, -(-S // 128) * 128)

    in_maps = []
    plans = []
    for c in range(NCORES):
        b = c // (NCORES // B)
        t0 = (c % (NCORES // B)) * per_batch
        toks = mask_indices[b, t0:t0 + per_batch]
        hmTf = np.ascontiguousarray(h_L[b][toks].T)      # [D, TOK]
        hmTc = np.ascontiguousarray(
            hmTf.reshape(ND, 128, TOK).transpose(1, 0, 2).reshape(
                128, ND * TOK)).astype(bf)
        nw = win // 128
        hLwc = np.zeros((NT, win, D), bf)
        ATwc = np.zeros((NT, win, 128), np.float32)
        for i in range(NT):
            ch = toks[i * 128:(i + 1) * 128]
            w0 = min(max(int(ch[0]) - r, 0), S - win)
            hLwc[i] = hLb[b][w0:w0 + win]
            if r > 0:
                mrow = t0 + i * 128
                v = valid[b, mrow:mrow + 128]            # [128, 2r]
                pc = posc[b, mrow:mrow + 128]
                jj, oo = np.nonzero(v)
                ATwc[i, pc[jj, oo] - w0, jj] = w[b, mrow + jj]
        ATwp = np.ascontiguousarray(
            ATwc.reshape(NT, nw, 128, 128).transpose(2, 0, 1, 3).reshape(
                128, NT * nw * 128)).astype(bf)
        in_maps.append({
            "hLw": hLwc, "ATw": ATwp, "hmT": hmTc,
            "W1q": W1qp, "W1l": W1lp, "W2q": W2qp, "W2l": W2lp,
            "Wr": Wrb, "br": brf, "b1": b1f, "b2": b2b, "sel": selb,
        })
        plans.append((b, toks, cnt[b, t0:t0 + per_batch] > 0))
    return win, in_maps, plans


def kernel(h_L, W_r, b_r, W1, b1, W2, b2, mask_indices, unmasked_indices,
           range_r):
    h_L = np.asarray(h_L, np.float32)
    mask_indices = np.asarray(mask_indices)
    unmasked_indices = np.asarray(unmasked_indices)
    assert h_L.shape == (B, S, D) and mask_indices.shape == (B, M)

    variant = os.environ.get("AMIP_VARIANT", "w1mask")
    win, in_maps, plans = _host_prep(
        h_L, np.asarray(W_r, np.float32), np.asarray(b_r, np.float32),
        np.asarray(W1, np.float32), np.asarray(b1, np.float32),
        np.asarray(W2, np.float32), np.asarray(b2, np.float32),
        mask_indices, unmasked_indices, range_r, variant)

    global _LAST_KEY
    _LAST_KEY = (win, variant)
    nc = get_nc(win, variant)
    try:
        res = run_bass_kernel_spmd(nc, in_maps, core_ids=list(range(NCORES)))
    except Exception:
        # transient device faults under the axon tunnel clear on retry
        import time as _time
        _time.sleep(5)
        res = run_bass_kernel_spmd(nc, in_maps, core_ids=list(range(NCORES)))

    full = np.zeros((B, S, D), np.float32)
    for c in range(NCORES):
        b, toks, has = plans[c]
        o = np.asarray(res.results[c]["out"]).astype(np.float32)
        full[b, toks[has]] = o[has]
    return full


# revision 14
# speedup vs baseline: 1.3845x; 1.0145x over previous
"""AMIP router kernel for 8 TRN2 NeuronCores — fp8 DoubleRow edition.

Sharding: data-parallel over tokens (as baseline): core c handles 512
tokens (half of batch c//2's masked set). Router/expert weights
replicated.

Speed comes from fp8e4 (e4m3) matmuls in MatmulPerfMode.DoubleRow: one
PE instruction contracts 2 k-tiles (256 rows) at 0.5 cycles per output
column. Accuracy is restored with hi/lo residual passes accumulated
into the same PSUM group:

  X @ W  ~=  Xq @ Wq  +  Xlo @ Wq  +  Xq @ Wlo
  (Xq = fp8(X), Xlo = fp8(X - Xq); W pre-scaled x64 so its fp8
   quantization stays out of the subnormal range; the 1/64 is folded
   into the PSUM-drain activation's scale.)

Device math per core (TOK=512 tokens):
  phase A : h_avgT windows (bf16 matmuls, unchanged from baseline),
            PSUM drained to Xq (ACT cast) + Xlo (DVE sub) fp8 tiles.
  gate    : expT[k,t] = exp(W_r^T h_mask + b_r) from bf16 h_mask
            (unnormalized; softmax denominator cancels in LayerNorm).
            Broadcast via one-hot selector matmul scaled by HGS=1/8 so
            the gated hidden stays inside fp8e4's +-240 range.
  phase B : per expert, 3 DoubleRow passes over 32 feature-chunk pairs
            into 8 PSUM banks; GELU drain (scale 1/64, bias b1) -> ht;
            DVE produces Hgq = fp8(ht*gb), Hgf = bf16(ht*gb),
            Hglo = fp8(Hgf - Hgq).
            Variant "w1mask" drops the W1lo pass on the h_avg feature
            half (13% of X variance -> ~1% extra noise, measured
            rel_err 1.6e-2 vs 8.7e-3 for "full"; gate is 2e-2).
  phase C : Y = sum_k Hg_k^T W2_k via 3 DoubleRow passes over 4
            DH-chunk pairs, + expT^T b2 (bf16, b2 host-scaled x8);
            drain scale 1/8; LayerNorm; bf16 output DMA (host casts
            back to f32).
  Mask-half pre-pass for the first NPRE experts fills the DMA-bound
  front (windows still streaming) with PE work, as in the baseline.

Hardware notes inherited from the baseline session:
- vector.tensor_tensor_reduce crashes the exec unit on silicon; keep
  sum-of-squares on ACT (Square + accum_out).
- transient NRT_EXEC_UNIT_UNRECOVERABLE faults under the axon tunnel
  clear on retry.
Validated on silicon this session: ACT f32/bf16->fp8e4 casts and DVE
mixed-dtype subtract are bit-exact vs ml_dtypes; DoubleRow with 3D
[128,2,N] APs matches the interpreter semantics.
"""

import os
import numpy as np
import ml_dtypes

import concourse.bass as bass
import concourse.bacc as bacc
import concourse.tile as tile
import concourse.mybir as mybir
from concourse.bass_utils import run_bass_kernel_spmd

BF16 = mybir.dt.bfloat16
F32 = mybir.dt.float32
FP8 = mybir.dt.float8e4
NPF8 = ml_dtypes.float8_e4m3
AF = mybir.ActivationFunctionType
ALU = mybir.AluOpType
DR = mybir.MatmulPerfMode.DoubleRow

B, S, D, K = 4, 2048, 4096, 8
M = S // 2
NCORES = 8
TOK = B * M // NCORES          # 512 tokens per core
DH = D // 4                    # 1024 expert hidden
TD = 2 * D                     # 8192 expert input
NT = TOK // 128                # 4 token chunks
NF = TD // 128                 # 64 X-feature chunks
ND = D // 128                  # 32 output-dim chunks
NH = DH // 128                 # 8 hidden chunks
NFP = NF // 2                  # 32 feature-chunk pairs
NHP = NH // 2                  # 4 hidden-chunk pairs
MFP0 = NFP // 2                # first mask-half feature pair (16)
NPRE = 2                       # experts pre-passed on the mask half

S1 = 64.0                      # W1 host prescale (pow2)
S2 = 64.0                      # W2 host prescale
HGS = 0.125                    # gate prescale (pow2), drain undoes it

_NC_CACHE = {}
_LAST_KEY = (384, "w1mask")


def _build_nc(win, variant):
    nw = win // 128
    full_lo = variant == "full"
    nlop = NFP if full_lo else NFP - MFP0   # W1lo pairs staged
    lo_off = 0 if full_lo else MFP0

    nc = bacc.Bacc("TRN2", target_bir_lowering=False, debug=False,
                   num_devices=NCORES)

    hLw = nc.dram_tensor("hLw", [NT, win, D], BF16, kind="ExternalInput")
    ATw = nc.dram_tensor("ATw", [128, NT * nw * 128], BF16, kind="ExternalInput")
    hmT = nc.dram_tensor("hmT", [128, ND * TOK], BF16, kind="ExternalInput")
    W1q = nc.dram_tensor("W1q", [K, NFP, 128, 2, DH], FP8, kind="ExternalInput")
    W1l = nc.dram_tensor("W1l", [K, nlop, 128, 2, DH], FP8, kind="ExternalInput")
    W2q = nc.dram_tensor("W2q", [K, NHP, 128, 2, D], FP8, kind="ExternalInput")
    W2l = nc.dram_tensor("W2l", [K, NHP, 128, 2, D], FP8, kind="ExternalInput")
    Wr = nc.dram_tensor("Wr", [128, ND * K], BF16, kind="ExternalInput")
    br = nc.dram_tensor("br", [K, 1], F32, kind="ExternalInput")
    b1 = nc.dram_tensor("b1", [128, K * NH], F32, kind="ExternalInput")
    b2 = nc.dram_tensor("b2", [K, D], BF16, kind="ExternalInput")
    sel = nc.dram_tensor("sel", [K, K * 128], BF16, kind="ExternalInput")
    out = nc.dram_tensor("out", [TOK, D], BF16, kind="ExternalOutput")

    debug = os.environ.get("AMIP_DEBUG") == "1"
    if debug:
        xq_d = nc.dram_tensor("xq_d", [128, NF, TOK], FP8, kind="ExternalOutput")
        xlo_d = nc.dram_tensor("xlo_d", [128, NF, TOK], FP8, kind="ExternalOutput")
        expT_d = nc.dram_tensor("expT_d", [K, TOK], BF16, kind="ExternalOutput")
        hgq_d = nc.dram_tensor("hgq_d", [128, K * NH, TOK], FP8,
                               kind="ExternalOutput")
        hgl_d = nc.dram_tensor("hgl_d", [128, K * NH, TOK], FP8,
                               kind="ExternalOutput")
        yraw_d = nc.dram_tensor("yraw_d", [TOK, D], BF16, kind="ExternalOutput")

    inv_s1 = 1.0 / S1
    inv_cdrain = 1.0 / (S2 * HGS)

    with tile.TileContext(nc) as tc:
        p_small = tc.alloc_tile_pool(name="small", bufs=1)
        # ---- small constants (sync ring, ahead of the W1 stream)
        wr_sb = p_small.tile([128, ND * K], BF16, name="wr_sb")
        nc.sync.dma_start(wr_sb[:], Wr[:, :])
        br_sb = p_small.tile([K, 1], F32, name="br_sb")
        nc.sync.dma_start(br_sb[:], br[:, :])
        b1_sb = p_small.tile([128, K * NH], F32, name="b1_sb")
        nc.sync.dma_start(b1_sb[:], b1[:, :])
        b2_sb = p_small.tile([K, D], BF16, name="b2_sb")
        nc.sync.dma_start(b2_sb[:], b2[:, :])
        sel_sb = p_small.tile([K, K * 128], BF16, name="sel_sb")
        nc.sync.dma_start(sel_sb[:], sel[:, :])
        expT = p_small.tile([K, TOK], BF16, name="expT")
        eps_sb = p_small.tile([128, 1], F32, name="eps_sb")
        nc.gpsimd.memset(eps_sb[:], 1e-5)
        gb_all = [p_small.tile([128, TOK], BF16, name="gball", tag="gball",
                               bufs=K) for _ in range(K)]

        # ---- h_mask chunks arrive on the vector ring so the sync ring can
        # start streaming W1 immediately (pre-pass feeds from it)
        p_xt = tc.alloc_tile_pool(name="xt", bufs=1)
        xq = p_xt.tile([128, NF, TOK], FP8, name="xq")
        xlo = p_xt.tile([128, NF, TOK], FP8, name="xlo")
        p_hm = tc.alloc_tile_pool(name="hmp", bufs=1)
        hm = p_hm.tile([128, ND * TOK], BF16, name="hm")
        for p in range(16):
            w = ND * TOK // 16
            nc.scalar.dma_start(hm[:, p * w:(p + 1) * w],
                                hmT[:, p * w:(p + 1) * w])

        # ---- h_mask half of X: fp8 cast + residual (per-piece order)
        for i in range(ND):
            src = hm[:, i * TOK:(i + 1) * TOK]
            nc.scalar.copy(xq[:, ND + i, :], src)
            nc.vector.tensor_sub(xlo[:, ND + i, :], src, xq[:, ND + i, :])

        # ---- mask-half pre-pass expert 0, then gate, then expert 1.
        # The gate PSUM tiles come from the same ring as the pre-pass banks
        # (one bank is recycled after expert 0's first drain), so the gate's
        # PE work slots between the two pre-pass experts without stalling.
        b0_partial = [[] for _ in range(NPRE)]
        p_w10 = tc.alloc_tile_pool(name="w1s0", bufs=8)
        psB0 = tc.alloc_tile_pool(name="psB0", bufs=8, space="PSUM")

        def prepass(kp):
            pst0 = [psB0.tile([128, TOK], F32, name="psb0", tag="psb0")
                    for _ in range(NH)]
            for fp in range(MFP0, NFP):
                slq = p_w10.tile([128, 2, DH], FP8, name="w10q", tag="w10")
                nc.sync.dma_start(slq[:], W1q[kp, fp, :, :, :])
                sll = p_w10.tile([128, 2, DH], FP8, name="w10l", tag="w10")
                nc.sync.dma_start(sll[:], W1l[kp, fp - lo_off, :, :, :])
                first = fp == MFP0
                last = fp == NFP - 1
                # full-bank-width matmuls: exactly one start=True per PSUM
                # bank (start poisons the whole 2KB bank on silicon, so
                # sub-bank regions must not interleave accumulation groups)
                for h in range(NH):
                    lq = slq[:, :, h * 128:(h + 1) * 128]
                    ll = sll[:, :, h * 128:(h + 1) * 128]
                    o = pst0[h][:]
                    rq = xq[:, 2 * fp:2 * fp + 2, :]
                    rl = xlo[:, 2 * fp:2 * fp + 2, :]
                    nc.tensor.matmul(o, lq, rq, perf_mode=DR,
                                     start=first, stop=False)
                    nc.tensor.matmul(o, lq, rl, perf_mode=DR,
                                     start=False, stop=False)
                    nc.tensor.matmul(o, ll, rq, perf_mode=DR,
                                     start=False, stop=last)
            for h in range(NH):
                t = p_xt.tile([128, TOK], BF16, name="b0p", tag="b0p",
                              bufs=NPRE * NH)
                nc.vector.tensor_copy(t[:], pst0[h][:])
                b0_partial[kp].append(t)

        prepass(0)

        # ---- gate: logitsT[k,t] over D, then exp (bf16 h_mask)
        ps_g = psB0.tile([128, TOK], F32, name="psb0", tag="psb0")
        for i in range(ND):
            nc.tensor.matmul(ps_g[0:K, :], wr_sb[:, i * K:(i + 1) * K],
                             hm[:, i * TOK:(i + 1) * TOK],
                             start=(i == 0), stop=(i == ND - 1))
        nc.scalar.activation(expT[:], ps_g[0:K, :], AF.Exp,
                             bias=br_sb[:, 0:1])
        # gate broadcast (sel entries = HGS, so gb = exp/8)
        for k in range(K):
            pg = psB0.tile([128, TOK], F32, name="psb0", tag="psb0")
            nc.tensor.matmul(pg[:], sel_sb[:, k * 128:(k + 1) * 128],
                             expT[:], start=True, stop=True)
            nc.scalar.copy(gb_all[k][:], pg[:])

        prepass(1)
        p_hm.release()  # gate consumed hm; casts into xq/xlo done

        # ---- phase A: windowed h_avgT -> Xq/Xlo fp8 (bf16 matmuls).
        # hLw slabs alternate between the scalar and vector rings so the
        # 12.6MB window stream keeps up with the PE.
        p_at = tc.alloc_tile_pool(name="atw", bufs=1)
        p_hl = tc.alloc_tile_pool(name="hlw", bufs=48)
        psA = tc.alloc_tile_pool(name="psA", bufs=8, space="PSUM")
        at_tile = p_at.tile([128, NT * nw * 128], BF16, name="at_tile")
        nc.scalar.dma_start(at_tile[:], ATw[:, :])
        atw = [at_tile[:, q * 128:(q + 1) * 128] for q in range(NT * nw)]
        slab_n = 0
        for dcg in range(ND // 8):    # groups of 8 d-chunks
            pts = [psA.tile([128, TOK], F32, name="psa", tag="psa")
                   for _ in range(8)]
            for i in range(NT):
                for s in range(nw):
                    slab = p_hl.tile([128, 1024], BF16, name="hlslab",
                                     tag="hl")
                    slab_n += 1
                    nc.scalar.dma_start(
                        slab[:], hLw[i, s * 128:(s + 1) * 128,
                                     dcg * 1024:(dcg + 1) * 1024])
                    for j in range(8):
                        nc.tensor.matmul(
                            pts[j][:, i * 128:(i + 1) * 128],
                            slab[:, j * 128:(j + 1) * 128],
                            atw[i * nw + s][:],
                            start=(s == 0), stop=(s == nw - 1))
            for j in range(8):
                dc = dcg * 8 + j
                nc.scalar.copy(xq[:, dc, :], pts[j][:])
                nc.vector.tensor_sub(xlo[:, dc, :], pts[j][:], xq[:, dc, :])
        p_at.release()
        p_hl.release()
        psA.release()

        # gated-hidden fp8 tiles, chunk-major so DoubleRow pairs are
        # adjacent: column block k*NH+h holds expert k's hidden chunk h
        p_hg = tc.alloc_tile_pool(name="hg", bufs=1)
        hgq = p_hg.tile([128, K * NH, TOK], FP8, name="hgq")
        hgl = p_hg.tile([128, K * NH, TOK], FP8, name="hgl")

        # ---- phase B main: per-expert 3-pass DoubleRow + drain
        p_w1 = tc.alloc_tile_pool(name="w1s", bufs=10)
        p_h = tc.alloc_tile_pool(name="htmp", bufs=4)
        psB = tc.alloc_tile_pool(name="psB", bufs=8, space="PSUM")
        p_w2 = tc.alloc_tile_pool(name="w2s", bufs=16)
        w2_slabs = {}

        def w2_fetch(dg, k, hp):
            slq = p_w2.tile([128, 2, 1024], FP8, name="w2q", tag="w2")
            nc.scalar.dma_start(
                slq[:], W2q[k, hp, :, :, dg * 1024:(dg + 1) * 1024])
            sll = p_w2.tile([128, 2, 1024], FP8, name="w2l", tag="w2")
            nc.scalar.dma_start(
                sll[:], W2l[k, hp, :, :, dg * 1024:(dg + 1) * 1024])
            w2_slabs[(dg, k, hp)] = (slq, sll)

        for k in range(K):
            pst = [psB.tile([128, TOK], F32, name="psb", tag="psb")
                   for _ in range(NH)]
            fp_hi = MFP0 if k < NPRE else NFP
            for fp in range(fp_hi):
                slq = p_w1.tile([128, 2, DH], FP8, name="w1q", tag="w1")
                nc.sync.dma_start(slq[:], W1q[k, fp, :, :, :])
                has_lo = full_lo or fp >= MFP0
                if has_lo:
                    sll = p_w1.tile([128, 2, DH], FP8, name="w1l", tag="w1")
                    nc.sync.dma_start(sll[:], W1l[k, fp - lo_off, :, :, :])
                first = fp == 0
                last = fp == fp_hi - 1
                for h in range(NH):
                    lq = slq[:, :, h * 128:(h + 1) * 128]
                    o = pst[h][:]
                    rq = xq[:, 2 * fp:2 * fp + 2, :]
                    rl = xlo[:, 2 * fp:2 * fp + 2, :]
                    nc.tensor.matmul(o, lq, rq, perf_mode=DR,
                                     start=first, stop=False)
                    nc.tensor.matmul(o, lq, rl, perf_mode=DR, start=False,
                                     stop=(last and not has_lo))
                    if has_lo:
                        ll = sll[:, :, h * 128:(h + 1) * 128]
                        nc.tensor.matmul(o, ll, rq, perf_mode=DR,
                                         start=False, stop=last)
            for h in range(NH):
                if k < NPRE:
                    nc.vector.tensor_add(pst[h][:], pst[h][:],
                                         b0_partial[k][h][:])
                col = k * NH + h
                ht = p_h.tile([128, TOK], BF16, name="ht", tag="ht")
                nc.scalar.activation(ht[:], pst[h][:], AF.Gelu,
                                     bias=b1_sb[:, col:col + 1],
                                     scale=inv_s1)
                nc.vector.tensor_mul(hgq[:, col, :], ht[:], gb_all[k][:])
                hgf = p_h.tile([128, TOK], BF16, name="hgf", tag="hgf")
                nc.vector.tensor_mul(hgf[:], ht[:], gb_all[k][:])
                nc.vector.tensor_sub(hgl[:, col, :], hgf[:], hgq[:, col, :])
            if k == K - 2:
                # prefetch the first W2 slab pairs so phase C starts hot
                for pre_i in range(6):
                    w2_fetch(0, pre_i // NHP, pre_i % NHP)

        if debug:
            nc.sync.dma_start(xq_d[:, :, :], xq[:])
            nc.sync.dma_start(xlo_d[:, :, :], xlo[:])
        p_xt.release()   # xq/xlo/b0 freed: phase C reuses the space
        p_w1.release()
        p_h.release()
        psB.release()
        if debug:
            nc.sync.dma_start(expT_d[:, :], expT[:])
            nc.sync.dma_start(hgq_d[:, :, :], hgq[:])
            nc.sync.dma_start(hgl_d[:, :, :], hgl[:])

        # ---- phase C: Y = sum_k Hg_k^T W2_k (3-pass DR) + expT^T b2;
        # LayerNorm; bf16 out. Single sweep over W2 (d-groups of 1024).
        p_y = tc.alloc_tile_pool(name="ysb", bufs=1)
        p_sq = tc.alloc_tile_pool(name="sq", bufs=4)
        p_stat = tc.alloc_tile_pool(name="stat", bufs=16)
        psC = tc.alloc_tile_pool(name="psC", bufs=8, space="PSUM")
        ysb = [p_y.tile([128, D], BF16, name="ysb", tag="ysbt", bufs=NT)
               for _ in range(NT)]
        sums = [p_stat.tile([128, 8], F32, name="sums", tag="sums")
                for _ in range(NT)]
        sumsq = [p_stat.tile([128, 8], F32, name="sumsq", tag="sumsq")
                 for _ in range(NT)]
        for dg in range(4):               # 1024-wide d column groups
            pst = {}
            # b2 bias term first: the sole start=True per bank, at full
            # bank width (start poisons the whole 2KB bank)
            for t in range(NT):
                for u in range(2):
                    p = psC.tile([128, 512], F32, name="psc", tag="psc")
                    pst[t * 2 + u] = p
                    c0 = dg * 1024 + u * 512
                    nc.tensor.matmul(p[:], expT[:, t * 128:(t + 1) * 128],
                                     b2_sb[:, c0:c0 + 512],
                                     start=True, stop=False)
            for k in range(K):
                for hp in range(NHP):
                    if (dg, k, hp) not in w2_slabs:
                        w2_fetch(dg, k, hp)
                    slq, sll = w2_slabs.pop((dg, k, hp))
                    # keep the fetch pipeline ~6 pairs ahead
                    ahead = k * NHP + hp + 6
                    if ahead < K * NHP:
                        w2_fetch(dg, ahead // NHP, ahead % NHP)
                    elif dg < 3:
                        a2 = ahead - K * NHP
                        w2_fetch(dg + 1, a2 // NHP, a2 % NHP)
                    last = k == K - 1 and hp == NHP - 1
                    col = k * NH + 2 * hp
                    for t in range(NT):
                        lq = hgq[:, col:col + 2, t * 128:(t + 1) * 128]
                        ll = hgl[:, col:col + 2, t * 128:(t + 1) * 128]
                        for u in range(2):
                            o = pst[t * 2 + u][:]
                            r = slice(u * 512, (u + 1) * 512)
                            nc.tensor.matmul(o, lq, slq[:, :, r],
                                             perf_mode=DR,
                                             start=False, stop=False)
                            nc.tensor.matmul(o, ll, slq[:, :, r],
                                             perf_mode=DR,
                                             start=False, stop=False)
                            nc.tensor.matmul(o, lq, sll[:, :, r],
                                             perf_mode=DR,
                                             start=False, stop=last)
            for t in range(NT):
                for u in range(2):
                    dc = dg * 2 + u
                    p = pst[t * 2 + u]
                    nc.scalar.activation(
                        ysb[t][:, dc * 512:(dc + 1) * 512], p[:],
                        AF.Identity, scale=inv_cdrain,
                        accum_out=sums[t][:, dc:dc + 1])
                    sq = p_sq.tile([128, 512], F32, name="sq", tag="sq")
                    nc.scalar.activation(
                        sq[:], p[:], AF.Square, scale=inv_cdrain,
                        accum_out=sumsq[t][:, dc:dc + 1])

        if debug:
            for t in range(NT):
                nc.sync.dma_start(yraw_d[t * 128:(t + 1) * 128, :],
                                  ysb[t][:])
        # ---- LayerNorm rows, then output (bf16).
        inv_d = 1.0 / D
        half = D // 2
        for t in range(NT):
            s1t = p_stat.tile([128, 1], F32, name="s1t", tag="st")
            nc.vector.tensor_reduce(s1t[:], sums[t][:, :],
                                    mybir.AxisListType.X, ALU.add)
            s2t = p_stat.tile([128, 1], F32, name="s2t", tag="st")
            nc.vector.tensor_reduce(s2t[:], sumsq[t][:, :],
                                    mybir.AxisListType.X, ALU.add)
            mu = p_stat.tile([128, 1], F32, name="mu", tag="st")
            nc.vector.tensor_scalar_mul(mu[:], s1t[:], inv_d)
            ex2 = p_stat.tile([128, 1], F32, name="ex2", tag="st")
            nc.vector.tensor_scalar_mul(ex2[:], s2t[:], inv_d)
            musq = p_stat.tile([128, 1], F32, name="musq", tag="st")
            nc.vector.tensor_mul(musq[:], mu[:], mu[:])
            var = p_stat.tile([128, 1], F32, name="var", tag="st")
            nc.vector.tensor_sub(var[:], ex2[:], musq[:])
            std = p_stat.tile([128, 1], F32, name="std", tag="st")
            nc.scalar.activation(std[:], var[:], AF.Sqrt,
                                 bias=eps_sb[:, 0:1])
            rstd = p_stat.tile([128, 1], F32, name="rstd", tag="st")
            nc.vector.reciprocal(rstd[:], std[:])
            nmr = p_stat.tile([128, 1], F32, name="nmr", tag="st")
            nc.vector.tensor_mul(nmr[:], mu[:], rstd[:])
            nc.vector.tensor_scalar_mul(nmr[:], nmr[:], -1.0)
            # left half on ACT: y*rstd - mu*rstd
            nc.scalar.activation(ysb[t][:, :half], ysb[t][:, :half],
                                 AF.Identity, bias=nmr[:, 0:1],
                                 scale=rstd[:, 0:1])
            nc.scalar.dma_start(out[t * 128:(t + 1) * 128, :half],
                                ysb[t][:, :half])
            # right half on DVE: (y - mu) * rstd
            nc.vector.tensor_scalar(ysb[t][:, half:], ysb[t][:, half:],
                                    mu[:], rstd[:],
                                    ALU.subtract, ALU.mult)
            nc.sync.dma_start(out[t * 128:(t + 1) * 128, half:],
                              ysb[t][:, half:])
        p_hg.release()
        p_w2.release()
        p_y.release()
        p_sq.release()
        p_stat.release()
        psC.release()
        psB0.release()
        p_w10.release()
        p_small.release()

    nc.compile()
    return nc


def get_nc(win=384, variant=None):
    if variant is None:
        variant = os.environ.get("AMIP_VARIANT", "w1mask")
    key = (win, variant)
    if key not in _NC_CACHE:
        _NC_CACHE[key] = _build_nc(win, variant)
    return _NC_CACHE[key]


def _host_prep(h_L, W_r, b_r, W1, b1, W2, b2, mask_indices, unmasked_indices,
               range_r, variant):
    """Index prep + quantization + sharding."""
    r = int(range_r)
    bf = ml_dtypes.bfloat16
    full_lo = variant == "full"

    is_un = np.zeros((B, S), bool)
    is_un[np.arange(B)[:, None], unmasked_indices] = True
    if r > 0:
        offs = np.concatenate([np.arange(-r, 0), np.arange(1, r + 1)])
        pos = mask_indices[:, :, None] + offs[None, None, :]      # [B,M,2r]
        inb = (pos >= 0) & (pos < S)
        posc = np.clip(pos, 0, S - 1)
        valid = inb & is_un[np.arange(B)[:, None, None], posc]
        cnt = valid.sum(-1)
        w = (1.0 / np.maximum(cnt, 1)).astype(np.float32)
    else:
        cnt = np.zeros((B, M), np.int64)

    # W1 DoubleRow packs: [K, NFP, 128, 2, DH]
    W1s = np.ascontiguousarray(W1, np.float32) * np.float32(S1)
    W1q8 = W1s.astype(NPF8)
    W1l8 = (W1s - W1q8.astype(np.float32)).astype(NPF8)
    del W1s

    def pack_w1(a):
        return np.ascontiguousarray(
            a.reshape(K, NFP, 2, 128, DH).transpose(0, 1, 3, 2, 4))
    W1qp = pack_w1(W1q8)
    W1lp = pack_w1(W1l8) if full_lo else pack_w1(W1l8)[:, MFP0:]
    del W1q8, W1l8

    W2s = np.ascontiguousarray(W2, np.float32) * np.float32(S2)
    W2q8 = W2s.astype(NPF8)
    W2l8 = (W2s - W2q8.astype(np.float32)).astype(NPF8)
    del W2s

    def pack_w2(a):
        return np.ascontiguousarray(
            a.reshape(K, NHP, 2, 128, D).transpose(0, 1, 3, 2, 4))
    W2qp = pack_w2(W2q8)
    W2lp = pack_w2(W2l8)
    del W2q8, W2l8

    b2b = np.ascontiguousarray(b2 * np.float32(S2 * HGS)).astype(bf)
    Wrb = np.ascontiguousarray(
        W_r.reshape(ND, 128, K).transpose(1, 0, 2).reshape(128, ND * K)
    ).astype(bf)
    brf = np.ascontiguousarray(b_r.reshape(K, 1)).astype(np.float32)
    b1f = np.ascontiguousarray(
        b1.reshape(K, NH, 128).transpose(2, 0, 1).reshape(128, K * NH)
    ).astype(np.float32)

    hLb = [np.ascontiguousarray(h_L[b]).astype(bf) for b in range(B)]

    selb = np.zeros((K, K * 128), bf)
    for k in range(K):
        selb[k, k * 128:(k + 1) * 128] = np.float32(HGS)

    per_batch = M // (NCORES // B)            # 512 tokens per core
    win = 256
    for b in range(B):
        for c in range(NCORES // B):
            toks = mask_indices[b, c * per_batch:(c + 1) * per_batch]
            for i in range(NT):
                ch = toks[i * 128:(i + 1) * 128]
                span = int(ch[-1]) + r - (int(ch[0]) - r) + 1
                win = max(win, -(-span // 128) * 128)
    win = min(win, -(-S // 128) * 128)

    in_maps = []
    plans = []
    for c in range(NCORES):
        b = c // (NCORES // B)
        t0 = (c % (NCORES // B)) * per_batch
        toks = mask_indices[b, t0:t0 + per_batch]
        hmTf = np.ascontiguousarray(h_L[b][toks].T)      # [D, TOK]
        hmTc = np.ascontiguousarray(
            hmTf.reshape(ND, 128, TOK).transpose(1, 0, 2).reshape(
                128, ND * TOK)).astype(bf)
        nw = win // 128
        hLwc = np.zeros((NT, win, D), bf)
        ATwc = np.zeros((NT, win, 128), np.float32)
        for i in range(NT):
            ch = toks[i * 128:(i + 1) * 128]
            w0 = min(max(int(ch[0]) - r, 0), S - win)
            hLwc[i] = hLb[b][w0:w0 + win]
            if r > 0:
                mrow = t0 + i * 128
                v = valid[b, mrow:mrow + 128]            # [128, 2r]
                pc = posc[b, mrow:mrow + 128]
                jj, oo = np.nonzero(v)
                ATwc[i, pc[jj, oo] - w0, jj] = w[b, mrow + jj]
        ATwp = np.ascontiguousarray(
            ATwc.reshape(NT, nw, 128, 128).transpose(2, 0, 1, 3).reshape(
                128, NT * nw * 128)).astype(bf)
        in_maps.append({
            "hLw": hLwc, "ATw": ATwp, "hmT": hmTc,
            "W1q": W1qp, "W1l": W1lp, "W2q": W2qp, "W2l": W2lp,
            "Wr": Wrb, "br": brf, "b1": b1f, "b2": b2b, "sel": selb,
        })
        plans.append((b, toks, cnt[b, t0:t0 + per_batch] > 0))
    return win, in_maps, plans


def kernel(h_L, W_r, b_r, W1, b1, W2, b2, mask_indices, unmasked_indices,
           range_r):
    h_L = np.asarray(h_L, np.float32)
    mask_indices = np.asarray(mask_indices)
    unmasked_indices = np.asarray(unmasked_indices)
    assert h_L.shape == (B, S, D) and mask_indices.shape == (B, M)

    variant = os.environ.get("AMIP_VARIANT", "w1mask")
    win, in_maps, plans = _host_prep(
        h_L, np.asarray(W_r, np.float32), np.asarray(b_r, np.float32),
        np.asarray(W1, np.float32), np.asarray(b1, np.float32),
        np.asarray(W2, np.float32), np.asarray(b2, np.float32),
        mask_indices, unmasked_indices, range_r, variant)

    global _LAST_KEY
    _LAST_KEY = (win, variant)
    nc = get_nc(win, variant)
    try:
        res = run_bass_kernel_spmd(nc, in_maps, core_ids=list(range(NCORES)))
    except Exception:
        # transient device faults under the axon tunnel clear on retry
        import time as _time
        _time.sleep(5)
        res = run_bass_kernel_spmd(nc, in_maps, core_ids=list(range(NCORES)))

    full = np.zeros((B, S, D), np.float32)
    for c in range(NCORES):
        b, toks, has = plans[c]
        o = np.asarray(res.results[c]["out"]).astype(np.float32)
        full[b, toks[has]] = o[has]
    return full
